# revision 8
# baseline (speedup 1.0000x reference)
"""GNN message-passing kernel for 8 Trainium2 NeuronCores.

Math (per reference):
  h   = relu(ef @ W1 + b1)                      [E, H]
  K   = (h @ W2 + b2).reshape(E, G, L)          per-edge [G, L] kernels
  t   = einsum('bnl,ne->bel', x, inc)           gather nodes->edges
  y   = einsum('egl,bel->beg', K, t)            per-edge matvec
  out = relu(einsum('ne,beg->bng', inc, y) + b_gc).reshape(B, N*G)

Distribution: shard the edge dim E across the 8 cores (2000 edges each,
padded to 2048 with zero-incidence edges).  All stages are edge-local;
the per-core scatter partials [B, N, G] are summed on the host, then
bias + relu applied.

v2 layout notes (vs v1):
  - mlp2/gather matmuls stream N=256 columns so the 128-col FWL weight
    load (~53ns) hides under the 107ns stream.
  - phase-2 scatter pads N 500->512 so lhsT tiles are 128 cols (FWL).
  - output staged bf16 (host upcasts + sums partials).
  - xT loaded in 4 slices so the first gather does not wait on 4MB.
"""

import numpy as np
import ml_dtypes

import concourse.bass as bass
from concourse import bacc
import concourse.mybir as mybir
import concourse.tile as tile
from concourse.bass_utils import run_bass_kernel_spmd
from concourse.masks import make_identity

B, N, E, L, G, F, H = 64, 500, 16000, 64, 64, 8, 128
NCORES = 8
ELR = E // NCORES       # 2000 real edges per core
EL = 2048               # padded; pad edges have zero incidence columns
ECH = 256               # edge chunk (phase 1)
NCH = EL // ECH         # 8 chunks
EHALF = 128             # matvec/transpose sub-chunk
NP = 125                # nodes per n-chunk (500 = 4*125)
NQ = 4                  # n-chunks
NPAD = 512              # padded node count for phase-2 FWL tiles
BG = B * G              # 4096
F32 = mybir.dt.float32
BF16 = mybir.dt.bfloat16
RELU = mybir.ActivationFunctionType.Relu
IDENT = mybir.ActivationFunctionType.Identity

_CACHE = {}
last_results = None     # BassKernelResults of the most recent run (for test.py)


def _build():
    nc = bacc.Bacc("TRN2", target_bir_lowering=False)
    xT_d = nc.declare_dram_parameter("xT", [N, B * L], BF16, isOutput=False)
    inc_d = nc.declare_dram_parameter("inc", [N, EL], BF16, isOutput=False)
    incT_d = nc.declare_dram_parameter("incT", [EL, NPAD], BF16, isOutput=False)
    efT_d = nc.declare_dram_parameter("efT", [F, EL], BF16, isOutput=False)
    W1_d = nc.declare_dram_parameter("W1", [F, H], BF16, isOutput=False)
    b1_d = nc.declare_dram_parameter("b1", [H, 1], F32, isOutput=False)
    W2_d = nc.declare_dram_parameter("W2", [H, G * L], BF16, isOutput=False)
    b2T_d = nc.declare_dram_parameter("b2T", [H, G * L // H], F32, isOutput=False)
    out_d = nc.declare_dram_parameter("out", [B, N, G], BF16, isOutput=True)
    y_d = nc.dram_tensor("Ystage", [EL, BG], BF16)

    with tile.TileContext(nc) as tc, tc.tile_pool(name="const", bufs=1) as cpool:
        with tc.tile_pool(name="h_ps", bufs=2, space="PSUM") as hps:
            # ---- persistent tiles ----
            W1_sb = cpool.tile([F, H], BF16)
            nc.sync.dma_start(out=W1_sb[:, :], in_=W1_d[:, :])
            b1_sb = cpool.tile([H, 1], F32)
            nc.sync.dma_start(out=b1_sb[:, :], in_=b1_d[:, :])
            W2_sb = cpool.tile([H, G * L], BF16)            # 8KB/part
            nc.sync.dma_start(out=W2_sb[:, :], in_=W2_d[:, :])
            b2T_sb = cpool.tile([H, G * L // H], F32)
            nc.sync.dma_start(out=b2T_sb[:, :], in_=b2T_d[:, :])
            efT_sb = cpool.tile([F, EL], BF16)
            nc.sync.dma_start(out=efT_sb[:, :], in_=efT_d[:, :])
            hT_sb = cpool.tile([H, EL], BF16)               # 4KB/part
            xT_sb = cpool.tile([NP, NQ, B * L], BF16)       # 32KB/part
            for q in range(NQ):
                nc.sync.dma_start(
                    out=xT_sb[:, q, :],
                    in_=xT_d[q * NP:(q + 1) * NP, :],
                )
            inc_sb = cpool.tile([NP, NQ, EL], BF16)         # 4KB/part
            for q in range(NQ):
                nc.sync.dma_start(
                    out=inc_sb[:, q, :],
                    in_=inc_d[q * NP:(q + 1) * NP, :],
                )

            # ---- mlp1: hT = relu(W1.T @ efT + b1), all edges upfront ----
            for c in range(4):
                ph = hps.tile([H, 512], F32)
                nc.tensor.matmul(
                    ph[:, :], lhsT=W1_sb[:, :],
                    rhs=efT_sb[:, c * 512:(c + 1) * 512],
                    start=True, stop=True,
                )
                nc.scalar.activation(
                    hT_sb[:, c * 512:(c + 1) * 512], ph[:, :], RELU,
                    bias=b1_sb[:, 0:1],
                )

        # ---- phase 1 ----
        with (
            tc.tile_pool(name="kt", bufs=1) as ktpool,
            tc.tile_pool(name="tt", bufs=2) as ttpool,
            tc.tile_pool(name="ycp", bufs=2) as ycppool,
            tc.tile_pool(name="yfin", bufs=1) as yfpool,
            tc.tile_pool(name="tid", bufs=1) as idpool,
            tc.tile_pool(name="mlp2_ps", bufs=2, space="PSUM") as mps,
            tc.tile_pool(name="gat_ps", bufs=2, space="PSUM") as gps,
            tc.tile_pool(name="mv_ps", bufs=2, space="PSUM") as vps,
            tc.tile_pool(name="tr_ps", bufs=2, space="PSUM") as tps,
        ):
            ident = idpool.tile([L, L], BF16)
            make_identity(nc, ident[:, :])
            for ch in range(NCH):
                e0 = ch * ECH
                # mlp2 -> kT[l, e, g] bf16 (+b2); strided drains buy a
                # contiguous 64-col lhsT for the per-edge matvec
                kT = ktpool.tile([L, ECH, G], BF16, tag="kt")
                for mc in range(32):
                    pm = mps.tile([H, ECH], F32, tag="m2")
                    nc.tensor.matmul(
                        pm[:, :], lhsT=W2_sb[:, mc * H:(mc + 1) * H],
                        rhs=hT_sb[:, e0:e0 + ECH], start=True, stop=True,
                    )
                    for par in (0, 1):
                        src = pm[par * 64:(par + 1) * 64, :]
                        dst = kT[:, :, 2 * mc + par]
                        bias = b2T_sb[par * 64:(par + 1) * 64, mc:mc + 1]
                        if mc % 2 == 0:
                            nc.scalar.activation(dst, src, IDENT, bias=bias)
                        else:
                            nc.vector.tensor_scalar_add(dst, src, bias)

                # gather -> tT[l, b, e] bf16
                tT = ttpool.tile([L, B, ECH], BF16, tag="tt")
                for bp in range(B // 2):
                    pg = gps.tile([2 * L, ECH], F32, tag="g")
                    for q in range(NQ):
                        nc.tensor.matmul(
                            pg[:, :],
                            lhsT=xT_sb[:, q, bp * 128:(bp + 1) * 128],
                            rhs=inc_sb[:, q, e0:e0 + ECH],
                            start=(q == 0), stop=(q == NQ - 1),
                        )
                    for par in (0, 1):
                        src = pg[par * 64:(par + 1) * 64, :]
                        dst = tT[:, 2 * bp + par, :]
                        if bp % 2 == 0:
                            nc.scalar.copy(dst, src)
                        else:
                            nc.vector.tensor_copy(dst, src)

                for hf in range(ECH // EHALF):
                    eh = hf * EHALF
                    # per-edge matvec: psum [g, 8e, b]
                    ycp = ycppool.tile([G, EHALF, B], BF16, tag="ycp")
                    for j in range(EHALF // 8):
                        pv = vps.tile([G, 8, B], F32, tag="mv")
                        for k in range(8):
                            er = eh + j * 8 + k
                            nc.tensor.matmul(
                                pv[:, k, :], lhsT=kT[:, er, :],
                                rhs=tT[:, :, er],
                                start=True, stop=True,
                            )
                        if j % 2 == 0:
                            nc.scalar.copy(ycp[:, j * 8:(j + 1) * 8, :],
                                           pv[:, :, :])
                        else:
                            nc.vector.tensor_copy(
                                ycp[:, j * 8:(j + 1) * 8, :], pv[:, :, :])

                    # PE transpose per b: [g, e] -> [e, g]
                    yfin = yfpool.tile([EHALF, B, G], BF16, tag="yf")
                    for b8 in range(B // 8):
                        pt = tps.tile([EHALF, 8, G], BF16, tag="tr")
                        for i in range(8):
                            b = b8 * 8 + i
                            nc.tensor.transpose(
                                pt[:, i, :], ycp[:, :, b], ident[:, :],
                            )
                        if b8 % 2 == 0:
                            nc.vector.tensor_copy(
                                yfin[:, b8 * 8:(b8 + 1) * 8, :], pt[:, :, :])
                        else:
                            nc.scalar.copy(
                                yfin[:, b8 * 8:(b8 + 1) * 8, :], pt[:, :, :])
                    nc.sync.dma_start(
                        out=y_d[e0 + eh:e0 + eh + EHALF, :],
                        in_=yfin[:, :, :],
                    )

        # ---- phase 2: scatter with PSUM accumulation over all edges ----
        NEC = EL // 128
        with (
            tc.tile_pool(name="p2c", bufs=1) as p2c,
            tc.tile_pool(name="p2rhs", bufs=6) as p2r,
            tc.tile_pool(name="acc_ps", bufs=8, space="PSUM") as aps,
        ):
            incT_sb = p2c.tile([128, NEC, NPAD], BF16)      # 16KB/part
            nc.sync.dma_start(
                out=incT_sb[:, :, :],
                in_=incT_d[:, :].rearrange("(c e) n -> e c n", c=NEC),
            )
            for nj in range(BG // 512):
                paccs = [aps.tile([128, 8, G], F32, tag="acc", name=f"acc{nj}_{m}")
                         for m in range(NQ)]
                for ec in range(NEC):
                    rt = p2r.tile([128, 512], BF16, tag="rhs")
                    nc.sync.dma_start(
                        out=rt[:, :],
                        in_=y_d[ec * 128:(ec + 1) * 128,
                                nj * 512:(nj + 1) * 512],
                    )
                    for m in range(NQ):
                        nc.tensor.matmul(
                            paccs[m][:, :, :],
                            lhsT=incT_sb[:, ec, m * 128:(m + 1) * 128],
                            rhs=rt[:, :],
                            start=(ec == 0), stop=(ec == NEC - 1),
                        )
                for m in range(NQ):
                    rows = min(N - m * 128, 128)
                    ot = p2r.tile([128, 8, G], BF16, tag="ostage",
                                  name=f"ost{nj}_{m}")
                    if m % 2 == 0:
                        nc.vector.tensor_copy(ot[:rows, :, :],
                                              paccs[m][:rows, :, :])
                    else:
                        nc.scalar.copy(ot[:rows, :, :], paccs[m][:rows, :, :])
                    nc.sync.dma_start(
                        out=out_d[nj * 8:(nj + 1) * 8,
                                  m * 128:m * 128 + rows, :].transpose(
                                      [1, 0, 2]),
                        in_=ot[:rows, :, :],
                    )
    nc.compile()
    return nc


def kernel(x, incidence, ef, W1, b1, W2, b2, b_gc):
    global last_results
    x = np.asarray(x, dtype=np.float32)
    incidence = np.asarray(incidence, dtype=np.float32)
    ef = np.asarray(ef, dtype=np.float32)
    W1 = np.asarray(W1, dtype=np.float32)
    b1 = np.asarray(b1, dtype=np.float32)
    W2 = np.asarray(W2, dtype=np.float32)
    b2 = np.asarray(b2, dtype=np.float32)
    b_gc = np.asarray(b_gc, dtype=np.float32)

    if "nc" not in _CACHE:
        _CACHE["nc"] = _build()
    nc = _CACHE["nc"]

    bf = ml_dtypes.bfloat16
    xT = np.ascontiguousarray(
        x.transpose(1, 0, 2).reshape(N, B * L)).astype(bf)
    inc_bf = incidence.astype(bf)
    incT_bf = np.ascontiguousarray(incidence.T).astype(bf)
    efT = np.ascontiguousarray(ef.T).astype(bf)
    b1c = np.ascontiguousarray(b1.reshape(H, 1))
    W2_bf = W2.astype(bf)
    b2T = np.ascontiguousarray(b2.reshape(G * L // H, H).T)

    pad = EL - ELR
    in_maps = []
    for c in range(NCORES):
        es = slice(c * ELR, (c + 1) * ELR)
        in_maps.append({
            "xT": xT,
            "inc": np.ascontiguousarray(
                np.pad(inc_bf[:, es], ((0, 0), (0, pad)))),
            "incT": np.ascontiguousarray(
                np.pad(incT_bf[es, :], ((0, pad), (0, NPAD - N)))),
            "efT": np.ascontiguousarray(
                np.pad(efT[:, es], ((0, 0), (0, pad)))),
            "W1": W1.astype(bf), "b1": b1c, "W2": W2_bf, "b2T": b2T,
        })

    import os
    trace = bool(int(os.environ.get("KERNEL_TRACE", "0")))
    last_results = run_bass_kernel_spmd(
        nc, in_maps, list(range(NCORES)), trace=trace)
    partial = np.zeros((B, N, G), np.float32)
    for r in last_results.results:
        partial += np.asarray(r["out"], dtype=np.float32)
    out = np.maximum(partial + b_gc.reshape(1, 1, G), 0.0)
    return out.reshape(B, N * G).astype(np.float32)


# revision 9
# speedup vs baseline: 1.0592x; 1.0592x over previous
"""GNN message-passing kernel for 8 Trainium2 NeuronCores.

Math (per reference):
  h   = relu(ef @ W1 + b1)                      [E, H]
  K   = (h @ W2 + b2).reshape(E, G, L)          per-edge [G, L] kernels
  t   = einsum('bnl,ne->bel', x, inc)           gather nodes->edges
  y   = einsum('egl,bel->beg', K, t)            per-edge matvec
  out = relu(einsum('ne,beg->bng', inc, y) + b_gc).reshape(B, N*G)

Distribution: shard E across the 8 cores (2000 edges each, padded to
2048 with zero-incidence edges); host sums the per-core scatter
partials, then bias + relu.

v4 notes:
  - mlp2 drains land contiguous in kTtmp[l,g,e]; the idle GpSimd engine
    re-layouts to kT[l,e,g] so the per-edge matvec gets contiguous
    64-col weight loads.
  - matvec packs edge PAIRS into the PE array via column tile_position
    (even edge -> psum partitions 0:64, odd edge -> 64:128), so weight
    loads for one half overlap matmuls on the other half.
  - y transposes split by edge parity into Y_even / Y_odd DRAM stages;
    phase-2 scatter runs two interleaved accumulation chains with
    host-deinterleaved incidence (incT[0::2], incT[1::2]).
  - phase-2 lhsT tiles padded to 128 cols; output staged bf16.
"""

import numpy as np
import ml_dtypes

import concourse.bass as bass
from concourse import bacc
import concourse.mybir as mybir
import concourse.tile as tile
from concourse.bass_utils import run_bass_kernel_spmd
from concourse.masks import make_identity

B, N, E, L, G, F, H = 64, 500, 16000, 64, 64, 8, 128
NCORES = 8
ELR = E // NCORES       # 2000 real edges per core
EL = 2048               # padded; pad edges have zero incidence columns
ECH = 256               # edge chunk (phase 1)
NCH = EL // ECH         # 8 chunks
NPR = ECH // 2          # 128 edge-pairs per chunk
NP = 125                # nodes per n-chunk (500 = 4*125)
NQ = 4                  # n-chunks
NPAD = 512              # padded node count for phase-2 FWL tiles
BG = B * G              # 4096
F32 = mybir.dt.float32
BF16 = mybir.dt.bfloat16
RELU = mybir.ActivationFunctionType.Relu
IDENT = mybir.ActivationFunctionType.Identity

_CACHE = {}
last_results = None     # BassKernelResults of the most recent run (for test.py)


def _build():
    nc = bacc.Bacc("TRN2", target_bir_lowering=False)
    xT_d = nc.declare_dram_parameter("xT", [N, B * L], BF16, isOutput=False)
    inc_d = nc.declare_dram_parameter("inc", [N, EL], BF16, isOutput=False)
    incTE_d = nc.declare_dram_parameter("incTE", [EL // 2, NPAD], BF16,
                                        isOutput=False)
    incTO_d = nc.declare_dram_parameter("incTO", [EL // 2, NPAD], BF16,
                                        isOutput=False)
    efT_d = nc.declare_dram_parameter("efT", [F, EL], BF16, isOutput=False)
    W1_d = nc.declare_dram_parameter("W1", [F, H], BF16, isOutput=False)
    b1_d = nc.declare_dram_parameter("b1", [H, 1], F32, isOutput=False)
    W2_d = nc.declare_dram_parameter("W2", [H, G * L], BF16, isOutput=False)
    b2T_d = nc.declare_dram_parameter("b2T", [H, G * L // H], F32, isOutput=False)
    out_d = nc.declare_dram_parameter("out", [B, N, G], BF16, isOutput=True)
    yE_d = nc.dram_tensor("YstageE", [EL // 2, BG], BF16)
    yO_d = nc.dram_tensor("YstageO", [EL // 2, BG], BF16)

    with tile.TileContext(nc) as tc, tc.tile_pool(name="const", bufs=1) as cpool:
        with tc.tile_pool(name="h_ps", bufs=2, space="PSUM") as hps:
            # ---- persistent tiles ----
            W1_sb = cpool.tile([F, H], BF16)
            nc.sync.dma_start(out=W1_sb[:, :], in_=W1_d[:, :])
            b1_sb = cpool.tile([H, 1], F32)
            nc.sync.dma_start(out=b1_sb[:, :], in_=b1_d[:, :])
            W2_sb = cpool.tile([H, G * L], BF16)            # 8KB/part
            nc.sync.dma_start(out=W2_sb[:, :], in_=W2_d[:, :])
            b2T_sb = cpool.tile([H, G * L // H], F32)
            nc.sync.dma_start(out=b2T_sb[:, :], in_=b2T_d[:, :])
            efT_sb = cpool.tile([F, EL], BF16)
            nc.sync.dma_start(out=efT_sb[:, :], in_=efT_d[:, :])
            hT_sb = cpool.tile([H, EL], BF16)               # 4KB/part
            xT_sb = cpool.tile([NP, NQ, B * L], BF16)       # 32KB/part
            for q in range(NQ):
                nc.sync.dma_start(
                    out=xT_sb[:, q, :],
                    in_=xT_d[q * NP:(q + 1) * NP, :],
                )

            # ---- mlp1: hT = relu(W1.T @ efT + b1), all edges upfront ----
            for c in range(4):
                ph = hps.tile([H, 512], F32)
                nc.tensor.matmul(
                    ph[:, :], lhsT=W1_sb[:, :],
                    rhs=efT_sb[:, c * 512:(c + 1) * 512],
                    start=True, stop=True,
                )
                nc.scalar.activation(
                    hT_sb[:, c * 512:(c + 1) * 512], ph[:, :], RELU,
                    bias=b1_sb[:, 0:1],
                )

        # ---- phase 1 ----
        with (
            tc.tile_pool(name="ktt", bufs=1) as kttpool,
            tc.tile_pool(name="kt", bufs=1) as ktpool,
            tc.tile_pool(name="tt", bufs=1) as ttpool,
            tc.tile_pool(name="inct", bufs=2) as incpool,
            tc.tile_pool(name="ycp", bufs=2) as ycppool,
            tc.tile_pool(name="yfin", bufs=1) as yfpool,
            tc.tile_pool(name="tid", bufs=1) as idpool,
            tc.tile_pool(name="mlp2_ps", bufs=2, space="PSUM") as mps,
            tc.tile_pool(name="gat_ps", bufs=2, space="PSUM") as gps,
            tc.tile_pool(name="mv_ps", bufs=2, space="PSUM") as vps,
            tc.tile_pool(name="tr_ps", bufs=2, space="PSUM") as tps,
        ):
            # identity blocks on both partition halves (for j=1 transposes)
            ident = idpool.tile([2 * L, L], BF16)
            make_identity(nc, ident[0:L, :])
            nc.vector.tensor_copy(ident[L:2 * L, :], ident[0:L, :])
            for ch in range(NCH):
                e0 = ch * ECH
                # mlp2 -> kTtmp[l, g, e] bf16 (+b2), contiguous drains
                kTtmp = kttpool.tile([L, G, ECH], BF16, tag="ktt")
                for mc in range(32):
                    pm = mps.tile([H, ECH], F32, tag="m2")
                    nc.tensor.matmul(
                        pm[:, :], lhsT=W2_sb[:, mc * H:(mc + 1) * H],
                        rhs=hT_sb[:, e0:e0 + ECH], start=True, stop=True,
                    )
                    for par in (0, 1):
                        src = pm[par * 64:(par + 1) * 64, :]
                        dst = kTtmp[:, 2 * mc + par, :]
                        bias = b2T_sb[par * 64:(par + 1) * 64, mc:mc + 1]
                        if mc % 2 == 0:
                            nc.scalar.activation(dst, src, IDENT, bias=bias)
                        else:
                            nc.vector.tensor_scalar_add(dst, src, bias)

                # GpSimd re-layout: kT[l, e, g] (contiguous matvec lhsT)
                kT = ktpool.tile([L, ECH, G], BF16, tag="kt")
                for h in (0, 1):
                    nc.gpsimd.tensor_copy(
                        kT[:, h * 128:(h + 1) * 128, :],
                        kTtmp[:, :, h * 128:(h + 1) * 128].transpose([0, 2, 1]),
                    )

                # gather -> tT[l, b, e] bf16
                inc_t = incpool.tile([NP, NQ, ECH], BF16, tag="inc")
                nc.sync.dma_start(
                    out=inc_t[:, :, :],
                    in_=inc_d[:, e0:e0 + ECH].rearrange("(q n) e -> n q e", q=NQ),
                )
                tT = ttpool.tile([L, B, ECH], BF16, tag="tt")
                for bp in range(B // 2):
                    pg = gps.tile([2 * L, ECH], F32, tag="g")
                    for q in range(NQ):
                        nc.tensor.matmul(
                            pg[:, :],
                            lhsT=xT_sb[:, q, bp * 128:(bp + 1) * 128],
                            rhs=inc_t[:, q, :],
                            start=(q == 0), stop=(q == NQ - 1),
                        )
                    for par in (0, 1):
                        src = pg[par * 64:(par + 1) * 64, :]
                        dst = tT[:, 2 * bp + par, :]
                        if bp % 2 == 0:
                            nc.scalar.copy(dst, src)
                        else:
                            nc.vector.tensor_copy(dst, src)

                # paired matvec: even edge -> psum rows 0:64 (col tile 0),
                # odd edge -> rows 64:128 (col tile 64); 8 pairs per bank
                ycp = ycppool.tile([2 * L, NPR, B], BF16, tag="ycp")
                for blk in range(NPR // 8):
                    pv = vps.tile([2 * L, 8, B], F32, tag="mv")
                    for k in range(8):
                        pr = blk * 8 + k
                        ee = e0 + 2 * pr
                        nc.tensor.matmul(
                            pv[0:64, k, :], lhsT=kT[:, 2 * pr, :],
                            rhs=tT[:, :, 2 * pr],
                            start=True, stop=True,
                        )
                        nc.tensor.matmul(
                            pv[64:128, k, :], lhsT=kT[:, 2 * pr + 1, :],
                            rhs=tT[:, :, 2 * pr + 1],
                            start=True, stop=True,
                        )
                    if blk % 2 == 0:
                        nc.scalar.copy(ycp[:, blk * 8:(blk + 1) * 8, :],
                                       pv[:, :, :])
                    else:
                        nc.vector.tensor_copy(
                            ycp[:, blk * 8:(blk + 1) * 8, :], pv[:, :, :])

                # PE transpose per (parity j, b): [g, pair] -> [pair, g]
                yfE = yfpool.tile([NPR, B, G], BF16, tag="yfE")
                yfO = yfpool.tile([NPR, B, G], BF16, tag="yfO")
                for j, yf in ((0, yfE), (1, yfO)):
                    for b8 in range(B // 8):
                        pt = tps.tile([NPR, 8, G], BF16, tag="tr")
                        for i in range(8):
                            b = b8 * 8 + i
                            nc.tensor.transpose(
                                pt[:, i, :],
                                ycp[j * 64:(j + 1) * 64, :, b],
                                ident[j * 64:(j + 1) * 64, :],
                            )
                        if (b8 + j) % 2 == 0:
                            nc.vector.tensor_copy(
                                yf[:, b8 * 8:(b8 + 1) * 8, :], pt[:, :, :])
                        else:
                            nc.scalar.copy(
                                yf[:, b8 * 8:(b8 + 1) * 8, :], pt[:, :, :])
                nc.sync.dma_start(
                    out=yE_d[ch * NPR:(ch + 1) * NPR, :], in_=yfE[:, :, :])
                nc.sync.dma_start(
                    out=yO_d[ch * NPR:(ch + 1) * NPR, :], in_=yfO[:, :, :])

        # ---- phase 2: scatter, two chains (even/odd edges), PSUM acc ----
        NEC = (EL // 2) // 128          # 8 pair-chunks of 128
        with (
            tc.tile_pool(name="p2c", bufs=1) as p2c,
            tc.tile_pool(name="p2rhs", bufs=6) as p2r,
            tc.tile_pool(name="acc_ps", bufs=8, space="PSUM") as aps,
        ):
            incTE_sb = p2c.tile([128, NEC, NPAD], BF16)     # 8KB/part
            nc.sync.dma_start(
                out=incTE_sb[:, :, :],
                in_=incTE_d[:, :].rearrange("(c e) n -> e c n", c=NEC),
            )
            incTO_sb = p2c.tile([128, NEC, NPAD], BF16)     # 8KB/part
            nc.sync.dma_start(
                out=incTO_sb[:, :, :],
                in_=incTO_d[:, :].rearrange("(c e) n -> e c n", c=NEC),
            )
            for nj in range(BG // 512):
                paccs = [aps.tile([128, 8, G], F32, tag="acc", name=f"acc{nj}_{m}")
                         for m in range(NQ)]
                for ec in range(NEC):
                    rtE = p2r.tile([128, 512], BF16, tag="rhsE")
                    nc.sync.dma_start(
                        out=rtE[:, :],
                        in_=yE_d[ec * 128:(ec + 1) * 128,
                                 nj * 512:(nj + 1) * 512],
                    )
                    rtO = p2r.tile([128, 512], BF16, tag="rhsO")
                    nc.sync.dma_start(
                        out=rtO[:, :],
                        in_=yO_d[ec * 128:(ec + 1) * 128,
                                 nj * 512:(nj + 1) * 512],
                    )
                    for m in range(NQ):
                        nc.tensor.matmul(
                            paccs[m][:, :, :],
                            lhsT=incTE_sb[:, ec, m * 128:(m + 1) * 128],
                            rhs=rtE[:, :],
                            start=(ec == 0), stop=False,
                        )
                        nc.tensor.matmul(
                            paccs[m][:, :, :],
                            lhsT=incTO_sb[:, ec, m * 128:(m + 1) * 128],
                            rhs=rtO[:, :],
                            start=False, stop=(ec == NEC - 1),
                        )
                for m in range(NQ):
                    rows = min(N - m * 128, 128)
                    ot = p2r.tile([128, 8, G], BF16, tag="ostage",
                                  name=f"ost{nj}_{m}")
                    if m % 2 == 0:
                        nc.vector.tensor_copy(ot[:rows, :, :],
                                              paccs[m][:rows, :, :])
                    else:
                        nc.scalar.copy(ot[:rows, :, :], paccs[m][:rows, :, :])
                    nc.sync.dma_start(
                        out=out_d[nj * 8:(nj + 1) * 8,
                                  m * 128:m * 128 + rows, :].transpose(
                                      [1, 0, 2]),
                        in_=ot[:rows, :, :],
                    )
    nc.compile()
    return nc


def kernel(x, incidence, ef, W1, b1, W2, b2, b_gc):
    global last_results
    x = np.asarray(x, dtype=np.float32)
    incidence = np.asarray(incidence, dtype=np.float32)
    ef = np.asarray(ef, dtype=np.float32)
    W1 = np.asarray(W1, dtype=np.float32)
    b1 = np.asarray(b1, dtype=np.float32)
    W2 = np.asarray(W2, dtype=np.float32)
    b2 = np.asarray(b2, dtype=np.float32)
    b_gc = np.asarray(b_gc, dtype=np.float32)

    if "nc" not in _CACHE:
        _CACHE["nc"] = _build()
    nc = _CACHE["nc"]

    bf = ml_dtypes.bfloat16
    xT = np.ascontiguousarray(
        x.transpose(1, 0, 2).reshape(N, B * L)).astype(bf)
    inc_bf = incidence.astype(bf)
    incT_bf = np.ascontiguousarray(incidence.T).astype(bf)
    efT = np.ascontiguousarray(ef.T).astype(bf)
    b1c = np.ascontiguousarray(b1.reshape(H, 1))
    W2_bf = W2.astype(bf)
    b2T = np.ascontiguousarray(b2.reshape(G * L // H, H).T)

    pad = EL - ELR
    in_maps = []
    for c in range(NCORES):
        es = slice(c * ELR, (c + 1) * ELR)
        incT_pad = np.pad(incT_bf[es, :], ((0, pad), (0, NPAD - N)))
        in_maps.append({
            "xT": xT,
            "inc": np.ascontiguousarray(
                np.pad(inc_bf[:, es], ((0, 0), (0, pad)))),
            "incTE": np.ascontiguousarray(incT_pad[0::2, :]),
            "incTO": np.ascontiguousarray(incT_pad[1::2, :]),
            "efT": np.ascontiguousarray(
                np.pad(efT[:, es], ((0, 0), (0, pad)))),
            "W1": W1.astype(bf), "b1": b1c, "W2": W2_bf, "b2T": b2T,
        })

    import os
    trace = bool(int(os.environ.get("KERNEL_TRACE", "0")))
    last_results = run_bass_kernel_spmd(
        nc, in_maps, list(range(NCORES)), trace=trace)
    partial = np.zeros((B, N, G), np.float32)
    for r in last_results.results:
        partial += np.asarray(r["out"], dtype=np.float32)
    out = np.maximum(partial + b_gc.reshape(1, 1, G), 0.0)
    return out.reshape(B, N * G).astype(np.float32)


# revision 14
# speedup vs baseline: 1.3152x; 1.2417x over previous
"""GNN message-passing kernel for 8 Trainium2 NeuronCores.

Math (per reference):
  h   = relu(ef @ W1 + b1)                      [E, H]
  K   = (h @ W2 + b2).reshape(E, G, L)          per-edge [G, L] kernels
  t   = einsum('bnl,ne->bel', x, inc)           gather nodes->edges
  y   = einsum('egl,bel->beg', K, t)            per-edge matvec
  out = relu(einsum('ne,beg->bng', inc, y) + b_gc).reshape(B, N*G)

Distribution: shard E across the 8 cores (2000 edges each, padded to
2048 with zero-incidence edges); host sums the per-core scatter
partials, then bias + relu.

v4 notes:
  - mlp2 drains land contiguous in kTtmp[l,g,e]; the idle GpSimd engine
    re-layouts to kT[l,e,g] so the per-edge matvec gets contiguous
    64-col weight loads.
  - matvec packs edge PAIRS into the PE array via column tile_position
    (even edge -> psum partitions 0:64, odd edge -> 64:128), so weight
    loads for one half overlap matmuls on the other half.
  - y transposes split by edge parity into Y_even / Y_odd DRAM stages;
    phase-2 scatter runs two interleaved accumulation chains with
    host-deinterleaved incidence (incT[0::2], incT[1::2]).
  - phase-2 lhsT tiles padded to 128 cols; output staged bf16.
"""

import numpy as np
import ml_dtypes

import concourse.bass as bass
from concourse import bacc
import concourse.mybir as mybir
import concourse.tile as tile
from concourse.bass_utils import run_bass_kernel_spmd
from concourse.masks import make_identity

B, N, E, L, G, F, H = 64, 500, 16000, 64, 64, 8, 128
NCORES = 8
ELR = E // NCORES       # 2000 real edges per core
EL = 2048               # padded; pad edges have zero incidence columns
ECH = 256               # edge chunk (phase 1)
NCH = EL // ECH         # 8 chunks
NPR = ECH // 2          # 128 edge-pairs per chunk
NP = 125                # nodes per n-chunk (500 = 4*125)
NQ = 4                  # n-chunks
NPAD = 512              # padded node count for phase-2 FWL tiles
BG = B * G              # 4096
F32 = mybir.dt.float32
BF16 = mybir.dt.bfloat16
RELU = mybir.ActivationFunctionType.Relu
IDENT = mybir.ActivationFunctionType.Identity

_CACHE = {}
last_results = None     # BassKernelResults of the most recent run (for test.py)


def _build():
    nc = bacc.Bacc("TRN2", target_bir_lowering=False)
    xT_d = nc.declare_dram_parameter("xT", [N, B * L], BF16, isOutput=False)
    inc_d = nc.declare_dram_parameter("inc", [N, EL], BF16, isOutput=False)
    incTE_d = nc.declare_dram_parameter("incTE", [EL // 2, NPAD], BF16,
                                        isOutput=False)
    incTO_d = nc.declare_dram_parameter("incTO", [EL // 2, NPAD], BF16,
                                        isOutput=False)
    efT_d = nc.declare_dram_parameter("efT", [F, EL], BF16, isOutput=False)
    W1_d = nc.declare_dram_parameter("W1", [F, H], BF16, isOutput=False)
    b1_d = nc.declare_dram_parameter("b1", [H, 1], F32, isOutput=False)
    W2_d = nc.declare_dram_parameter("W2", [H, G * L], BF16, isOutput=False)
    b2T_d = nc.declare_dram_parameter("b2T", [H, G * L // H], F32, isOutput=False)
    out_d = nc.declare_dram_parameter("out", [B, N, G], BF16, isOutput=True)
    yE_d = nc.dram_tensor("YstageE", [EL // 2, BG], BF16)
    yO_d = nc.dram_tensor("YstageO", [EL // 2, BG], BF16)

    with tile.TileContext(nc) as tc, tc.tile_pool(name="const", bufs=1) as cpool:
        with tc.tile_pool(name="h_ps", bufs=2, space="PSUM") as hps:
            # ---- persistent tiles ----
            W1_sb = cpool.tile([F, H], BF16)
            nc.sync.dma_start(out=W1_sb[:, :], in_=W1_d[:, :])
            b1_sb = cpool.tile([H, 1], F32)
            nc.sync.dma_start(out=b1_sb[:, :], in_=b1_d[:, :])
            W2_sb = cpool.tile([H, G * L], BF16)            # 8KB/part
            nc.sync.dma_start(out=W2_sb[:, :], in_=W2_d[:, :])
            b2T_sb = cpool.tile([H, G * L // H], F32)
            nc.sync.dma_start(out=b2T_sb[:, :], in_=b2T_d[:, :])
            efT_sb = cpool.tile([F, EL], BF16)
            nc.sync.dma_start(out=efT_sb[:, :], in_=efT_d[:, :])
            hT_sb = cpool.tile([H, EL], BF16)               # 4KB/part
            xT_sb = cpool.tile([NP, NQ, B * L], BF16)       # 32KB/part
            for q in range(NQ):
                nc.sync.dma_start(
                    out=xT_sb[:, q, :],
                    in_=xT_d[q * NP:(q + 1) * NP, :],
                )

            # ---- mlp1: hT = relu(W1.T @ efT + b1), all edges upfront ----
            for c in range(4):
                ph = hps.tile([H, 512], F32)
                nc.tensor.matmul(
                    ph[:, :], lhsT=W1_sb[:, :],
                    rhs=efT_sb[:, c * 512:(c + 1) * 512],
                    start=True, stop=True,
                )
                nc.scalar.activation(
                    hT_sb[:, c * 512:(c + 1) * 512], ph[:, :], RELU,
                    bias=b1_sb[:, 0:1],
                )

        # ---- phase 1 ----
        with (
            tc.tile_pool(name="ktt", bufs=2) as kttpool,
            tc.tile_pool(name="tt", bufs=1) as ttpool,
            tc.tile_pool(name="inct", bufs=2) as incpool,
            tc.tile_pool(name="ycp", bufs=1) as ycppool,
            tc.tile_pool(name="yfin", bufs=1) as yfpool,
            tc.tile_pool(name="tid", bufs=1) as idpool,
            tc.tile_pool(name="mlp2_ps", bufs=2, space="PSUM") as mps,
            tc.tile_pool(name="gat_ps", bufs=2, space="PSUM") as gps,
            tc.tile_pool(name="mv_ps", bufs=2, space="PSUM") as vps,
            tc.tile_pool(name="tr_ps", bufs=2, space="PSUM") as tps,
        ):
            # identity blocks on both partition halves (for j=1 transposes)
            ident = idpool.tile([2 * L, L], BF16)
            make_identity(nc, ident[0:L, :])
            nc.vector.tensor_copy(ident[L:2 * L, :], ident[0:L, :])
            for ch in range(NCH):
                e0 = ch * ECH
                # mlp2 -> kTtmp[l, g, e] bf16 (+b2), contiguous drains
                kTtmp = kttpool.tile([L, G, ECH], BF16, tag="ktt")
                for mc in range(32):
                    pm = mps.tile([H, ECH], F32, tag="m2")
                    nc.tensor.matmul(
                        pm[:, :], lhsT=W2_sb[:, mc * H:(mc + 1) * H],
                        rhs=hT_sb[:, e0:e0 + ECH], start=True, stop=True,
                    )
                    for par in (0, 1):
                        src = pm[par * 64:(par + 1) * 64, :]
                        dst = kTtmp[:, 2 * mc + par, :]
                        bias = b2T_sb[par * 64:(par + 1) * 64, mc:mc + 1]
                        if mc % 2 == 0:
                            nc.scalar.activation(dst, src, IDENT, bias=bias)
                        else:
                            nc.vector.tensor_scalar_add(dst, src, bias)

                # gather -> tT[l, b, e] bf16
                inc_t = incpool.tile([NP, NQ, ECH], BF16, tag="inc")
                nc.sync.dma_start(
                    out=inc_t[:, :, :],
                    in_=inc_d[:, e0:e0 + ECH].rearrange("(q n) e -> n q e", q=NQ),
                )
                tT = ttpool.tile([L, B, ECH], BF16, tag="tt")
                for bp in range(B // 2):
                    pg = gps.tile([2 * L, ECH], F32, tag="g")
                    for q in range(NQ):
                        nc.tensor.matmul(
                            pg[:, :],
                            lhsT=xT_sb[:, q, bp * 128:(bp + 1) * 128],
                            rhs=inc_t[:, q, :],
                            start=(q == 0), stop=(q == NQ - 1),
                        )
                    for par in (0, 1):
                        src = pg[par * 64:(par + 1) * 64, :]
                        dst = tT[:, 2 * bp + par, :]
                        if bp % 2 == 0:
                            nc.scalar.copy(dst, src)
                        else:
                            nc.vector.tensor_copy(dst, src)

                # paired matvec: even edge -> psum rows 0:64 (col tile 0),
                # odd edge -> rows 64:128 (col tile 64); 8 pairs per bank
                ycp = ycppool.tile([2 * L, NPR, B], BF16, tag="ycp")
                for blk in range(NPR // 8):
                    pv = vps.tile([2 * L, 8, B], F32, tag="mv")
                    for k in range(8):
                        pr = blk * 8 + k
                        ee = e0 + 2 * pr
                        nc.tensor.matmul(
                            pv[0:64, k, :], lhsT=kTtmp[:, :, 2 * pr],
                            rhs=tT[:, :, 2 * pr],
                            start=True, stop=True,
                        )
                        nc.tensor.matmul(
                            pv[64:128, k, :], lhsT=kTtmp[:, :, 2 * pr + 1],
                            rhs=tT[:, :, 2 * pr + 1],
                            start=True, stop=True,
                        )
                    if blk % 2 == 0:
                        nc.scalar.copy(ycp[:, blk * 8:(blk + 1) * 8, :],
                                       pv[:, :, :])
                    else:
                        nc.vector.tensor_copy(
                            ycp[:, blk * 8:(blk + 1) * 8, :], pv[:, :, :])

                # PE transpose per (parity j, b): [g, pair] -> [pair, g]
                yfE = yfpool.tile([NPR, B, G], BF16, tag="yfE")
                yfO = yfpool.tile([NPR, B, G], BF16, tag="yfO")
                for j, yf in ((0, yfE), (1, yfO)):
                    for b8 in range(B // 8):
                        pt = tps.tile([NPR, 8, G], BF16, tag="tr")
                        for i in range(8):
                            b = b8 * 8 + i
                            nc.tensor.transpose(
                                pt[:, i, :],
                                ycp[j * 64:(j + 1) * 64, :, b],
                                ident[j * 64:(j + 1) * 64, :],
                            )
                        if (b8 + j) % 2 == 0:
                            nc.vector.tensor_copy(
                                yf[:, b8 * 8:(b8 + 1) * 8, :], pt[:, :, :])
                        else:
                            nc.scalar.copy(
                                yf[:, b8 * 8:(b8 + 1) * 8, :], pt[:, :, :])
                nc.sync.dma_start(
                    out=yE_d[ch * NPR:(ch + 1) * NPR, :], in_=yfE[:, :, :])
                nc.sync.dma_start(
                    out=yO_d[ch * NPR:(ch + 1) * NPR, :], in_=yfO[:, :, :])

        # ---- phase 2: scatter, two chains (even/odd edges), PSUM acc ----
        NEC = (EL // 2) // 128          # 8 pair-chunks of 128
        with (
            tc.tile_pool(name="p2c", bufs=1) as p2c,
            tc.tile_pool(name="p2rhs", bufs=6) as p2r,
            tc.tile_pool(name="acc_ps", bufs=8, space="PSUM") as aps,
        ):
            incTE_sb = p2c.tile([128, NEC, NPAD], BF16)     # 8KB/part
            nc.sync.dma_start(
                out=incTE_sb[:, :, :],
                in_=incTE_d[:, :].rearrange("(c e) n -> e c n", c=NEC),
            )
            incTO_sb = p2c.tile([128, NEC, NPAD], BF16)     # 8KB/part
            nc.sync.dma_start(
                out=incTO_sb[:, :, :],
                in_=incTO_d[:, :].rearrange("(c e) n -> e c n", c=NEC),
            )
            for nj in range(BG // 512):
                paccs = [aps.tile([128, 8, G], F32, tag="acc", name=f"acc{nj}_{m}")
                         for m in range(NQ)]
                for ec in range(NEC):
                    rtE = p2r.tile([128, 512], BF16, tag="rhsE")
                    nc.sync.dma_start(
                        out=rtE[:, :],
                        in_=yE_d[ec * 128:(ec + 1) * 128,
                                 nj * 512:(nj + 1) * 512],
                    )
                    rtO = p2r.tile([128, 512], BF16, tag="rhsO")
                    nc.sync.dma_start(
                        out=rtO[:, :],
                        in_=yO_d[ec * 128:(ec + 1) * 128,
                                 nj * 512:(nj + 1) * 512],
                    )
                    for m in range(NQ):
                        nc.tensor.matmul(
                            paccs[m][:, :, :],
                            lhsT=incTE_sb[:, ec, m * 128:(m + 1) * 128],
                            rhs=rtE[:, :],
                            start=(ec == 0), stop=False,
                        )
                        nc.tensor.matmul(
                            paccs[m][:, :, :],
                            lhsT=incTO_sb[:, ec, m * 128:(m + 1) * 128],
                            rhs=rtO[:, :],
                            start=False, stop=(ec == NEC - 1),
                        )
                for m in range(NQ):
                    rows = min(N - m * 128, 128)
                    ot = p2r.tile([128, 8, G], BF16, tag="ostage",
                                  name=f"ost{nj}_{m}")
                    if m % 2 == 0:
                        nc.vector.tensor_copy(ot[:rows, :, :],
                                              paccs[m][:rows, :, :])
                    else:
                        nc.scalar.copy(ot[:rows, :, :], paccs[m][:rows, :, :])
                    nc.sync.dma_start(
                        out=out_d[nj * 8:(nj + 1) * 8,
                                  m * 128:m * 128 + rows, :].transpose(
                                      [1, 0, 2]),
                        in_=ot[:rows, :, :],
                    )
    nc.compile()
    return nc


def kernel(x, incidence, ef, W1, b1, W2, b2, b_gc):
    global last_results
    x = np.asarray(x, dtype=np.float32)
    incidence = np.asarray(incidence, dtype=np.float32)
    ef = np.asarray(ef, dtype=np.float32)
    W1 = np.asarray(W1, dtype=np.float32)
    b1 = np.asarray(b1, dtype=np.float32)
    W2 = np.asarray(W2, dtype=np.float32)
    b2 = np.asarray(b2, dtype=np.float32)
    b_gc = np.asarray(b_gc, dtype=np.float32)

    if "nc" not in _CACHE:
        _CACHE["nc"] = _build()
    nc = _CACHE["nc"]

    bf = ml_dtypes.bfloat16
    xT = np.ascontiguousarray(
        x.transpose(1, 0, 2).reshape(N, B * L)).astype(bf)
    inc_bf = incidence.astype(bf)
    incT_bf = np.ascontiguousarray(incidence.T).astype(bf)
    efT = np.ascontiguousarray(ef.T).astype(bf)
    b1c = np.ascontiguousarray(b1.reshape(H, 1))
    W2_bf = W2.astype(bf)
    b2T = np.ascontiguousarray(b2.reshape(G * L // H, H).T)

    pad = EL - ELR
    in_maps = []
    for c in range(NCORES):
        es = slice(c * ELR, (c + 1) * ELR)
        incT_pad = np.pad(incT_bf[es, :], ((0, pad), (0, NPAD - N)))
        in_maps.append({
            "xT": xT,
            "inc": np.ascontiguousarray(
                np.pad(inc_bf[:, es], ((0, 0), (0, pad)))),
            "incTE": np.ascontiguousarray(incT_pad[0::2, :]),
            "incTO": np.ascontiguousarray(incT_pad[1::2, :]),
            "efT": np.ascontiguousarray(
                np.pad(efT[:, es], ((0, 0), (0, pad)))),
            "W1": W1.astype(bf), "b1": b1c, "W2": W2_bf, "b2T": b2T,
        })

    import os
    trace = bool(int(os.environ.get("KERNEL_TRACE", "0")))
    last_results = run_bass_kernel_spmd(
        nc, in_maps, list(range(NCORES)), trace=trace)
    partial = np.zeros((B, N, G), np.float32)
    for r in last_results.results:
        partial += np.asarray(r["out"], dtype=np.float32)
    out = np.maximum(partial + b_gc.reshape(1, 1, G), 0.0)
    return out.reshape(B, N * G).astype(np.float32)


# revision 17
# speedup vs baseline: 1.3259x; 1.0082x over previous
"""GNN message-passing kernel for 8 Trainium2 NeuronCores.

Math (per reference):
  h   = relu(ef @ W1 + b1)                      [E, H]
  K   = (h @ W2 + b2).reshape(E, G, L)          per-edge [G, L] kernels
  t   = einsum('bnl,ne->bel', x, inc)           gather nodes->edges
  y   = einsum('egl,bel->beg', K, t)            per-edge matvec
  out = relu(einsum('ne,beg->bng', inc, y) + b_gc).reshape(B, N*G)

Distribution: shard E across the 8 cores (2000 edges each, padded to
2048 with zero-incidence edges); host sums the per-core scatter
partials, then bias + relu.

v4 notes:
  - mlp2 drains land contiguous in kTtmp[l,g,e]; the idle GpSimd engine
    re-layouts to kT[l,e,g] so the per-edge matvec gets contiguous
    64-col weight loads.
  - matvec packs edge PAIRS into the PE array via column tile_position
    (even edge -> psum partitions 0:64, odd edge -> 64:128), so weight
    loads for one half overlap matmuls on the other half.
  - y transposes split by edge parity into Y_even / Y_odd DRAM stages;
    phase-2 scatter runs two interleaved accumulation chains with
    host-deinterleaved incidence (incT[0::2], incT[1::2]).
  - phase-2 lhsT tiles padded to 128 cols; output staged bf16.
"""

import numpy as np
import ml_dtypes

import concourse.bass as bass
from concourse import bacc
import concourse.mybir as mybir
import concourse.tile as tile
from concourse.bass_utils import run_bass_kernel_spmd
from concourse.masks import make_identity

B, N, E, L, G, F, H = 64, 500, 16000, 64, 64, 8, 128
NCORES = 8
ELR = E // NCORES       # 2000 real edges per core
EL = 2048               # padded; pad edges have zero incidence columns
ECH = 256               # edge chunk (phase 1)
NCH = EL // ECH         # 8 chunks
NPR = ECH // 2          # 128 edge-pairs per chunk
NP = 125                # nodes per n-chunk (500 = 4*125)
NQ = 4                  # n-chunks
NPAD = 512              # padded node count for phase-2 FWL tiles
BG = B * G              # 4096
F32 = mybir.dt.float32
BF16 = mybir.dt.bfloat16
RELU = mybir.ActivationFunctionType.Relu
IDENT = mybir.ActivationFunctionType.Identity

_CACHE = {}
last_results = None     # BassKernelResults of the most recent run (for test.py)


def _build():
    nc = bacc.Bacc("TRN2", target_bir_lowering=False)
    xT_d = nc.declare_dram_parameter("xT", [N, B * L], BF16, isOutput=False)
    inc_d = nc.declare_dram_parameter("inc", [N, EL], BF16, isOutput=False)
    incTE_d = nc.declare_dram_parameter("incTE", [EL // 2, NPAD], BF16,
                                        isOutput=False)
    incTO_d = nc.declare_dram_parameter("incTO", [EL // 2, NPAD], BF16,
                                        isOutput=False)
    efT_d = nc.declare_dram_parameter("efT", [F, EL], BF16, isOutput=False)
    W1_d = nc.declare_dram_parameter("W1", [F, H], BF16, isOutput=False)
    b1_d = nc.declare_dram_parameter("b1", [H, 1], F32, isOutput=False)
    W2_d = nc.declare_dram_parameter("W2", [H, G * L], BF16, isOutput=False)
    b2T_d = nc.declare_dram_parameter("b2T", [H, G * L // H], F32, isOutput=False)
    out_d = nc.declare_dram_parameter("out", [B, N, G], BF16, isOutput=True)
    yE_d = nc.dram_tensor("YstageE", [EL // 2, BG], BF16)
    yO_d = nc.dram_tensor("YstageO", [EL // 2, BG], BF16)

    with tile.TileContext(nc) as tc, tc.tile_pool(name="const", bufs=1) as cpool:
        with tc.tile_pool(name="h_ps", bufs=2, space="PSUM") as hps:
            # ---- persistent tiles ----
            W1_sb = cpool.tile([F, H], BF16)
            nc.sync.dma_start(out=W1_sb[:, :], in_=W1_d[:, :])
            b1_sb = cpool.tile([H, 1], F32)
            nc.sync.dma_start(out=b1_sb[:, :], in_=b1_d[:, :])
            W2_sb = cpool.tile([H, G * L], BF16)            # 8KB/part
            nc.sync.dma_start(out=W2_sb[:, :], in_=W2_d[:, :])
            b2T_sb = cpool.tile([H, G * L // H], F32)
            nc.sync.dma_start(out=b2T_sb[:, :], in_=b2T_d[:, :])
            efT_sb = cpool.tile([F, EL], BF16)
            nc.sync.dma_start(out=efT_sb[:, :], in_=efT_d[:, :])
            hT_sb = cpool.tile([H, EL], BF16)               # 4KB/part
            xT_sb = cpool.tile([NP, NQ, B * L], BF16)       # 32KB/part
            inc0_sb = cpool.tile([NP, NQ, ECH], BF16)       # chunk-0 inc early
            nc.sync.dma_start(
                out=inc0_sb[:, :, :],
                in_=inc_d[:, 0:ECH].rearrange("(q n) e -> n q e", q=NQ),
            )
            for q in range(NQ):
                nc.sync.dma_start(
                    out=xT_sb[:, q, :],
                    in_=xT_d[q * NP:(q + 1) * NP, :],
                )

            # ---- mlp1: hT = relu(W1.T @ efT + b1), all edges upfront ----
            for c in range(4):
                ph = hps.tile([H, 512], F32)
                nc.tensor.matmul(
                    ph[:, :], lhsT=W1_sb[:, :],
                    rhs=efT_sb[:, c * 512:(c + 1) * 512],
                    start=True, stop=True,
                )
                nc.scalar.activation(
                    hT_sb[:, c * 512:(c + 1) * 512], ph[:, :], RELU,
                    bias=b1_sb[:, 0:1],
                )

        # ---- phase 1 ----
        with (
            tc.tile_pool(name="ktt", bufs=2) as kttpool,
            tc.tile_pool(name="tt", bufs=1) as ttpool,
            tc.tile_pool(name="inct", bufs=2) as incpool,
            tc.tile_pool(name="ycp", bufs=1) as ycppool,
            tc.tile_pool(name="yfin", bufs=1) as yfpool,
            tc.tile_pool(name="tid", bufs=1) as idpool,
            tc.tile_pool(name="mlp2_ps", bufs=2, space="PSUM") as mps,
            tc.tile_pool(name="gat_ps", bufs=2, space="PSUM") as gps,
            tc.tile_pool(name="mv_ps", bufs=2, space="PSUM") as vps,
            tc.tile_pool(name="tr_ps", bufs=2, space="PSUM") as tps,
        ):
            # identity blocks on both partition halves (for j=1 transposes)
            ident = idpool.tile([2 * L, L], BF16)
            make_identity(nc, ident[0:L, :])
            nc.vector.tensor_copy(ident[L:2 * L, :], ident[0:L, :])
            for ch in range(NCH):
                e0 = ch * ECH
                # mlp2 -> kTtmp[l, g, e] bf16 (+b2), contiguous drains
                kTtmp = kttpool.tile([L, G, ECH], BF16, tag="ktt")
                for mc in range(32):
                    pm = mps.tile([H, ECH], F32, tag="m2")
                    nc.tensor.matmul(
                        pm[:, :], lhsT=W2_sb[:, mc * H:(mc + 1) * H],
                        rhs=hT_sb[:, e0:e0 + ECH], start=True, stop=True,
                    )
                    for par in (0, 1):
                        src = pm[par * 64:(par + 1) * 64, :]
                        dst = kTtmp[:, 2 * mc + par, :]
                        bias = b2T_sb[par * 64:(par + 1) * 64, mc:mc + 1]
                        if mc % 2 == 0:
                            nc.scalar.activation(dst, src, IDENT, bias=bias)
                        else:
                            nc.vector.tensor_scalar_add(dst, src, bias)

                # gather -> tT[l, b, e] bf16
                if ch == 0:
                    inc_t = inc0_sb
                else:
                    inc_t = incpool.tile([NP, NQ, ECH], BF16, tag="inc")
                    nc.sync.dma_start(
                        out=inc_t[:, :, :],
                        in_=inc_d[:, e0:e0 + ECH].rearrange(
                            "(q n) e -> n q e", q=NQ),
                    )
                tT = ttpool.tile([L, B, ECH], BF16, tag="tt")
                for bp in range(B // 2):
                    pg = gps.tile([2 * L, ECH], F32, tag="g")
                    for q in range(NQ):
                        nc.tensor.matmul(
                            pg[:, :],
                            lhsT=xT_sb[:, q, bp * 128:(bp + 1) * 128],
                            rhs=inc_t[:, q, :],
                            start=(q == 0), stop=(q == NQ - 1),
                        )
                    for par in (0, 1):
                        src = pg[par * 64:(par + 1) * 64, :]
                        dst = tT[:, 2 * bp + par, :]
                        if bp % 2 == 0:
                            nc.scalar.copy(dst, src)
                        else:
                            nc.vector.tensor_copy(dst, src)

                # paired matvec: even edge -> psum rows 0:64 (col tile 0),
                # odd edge -> rows 64:128 (col tile 64); 8 pairs per bank
                ycp = ycppool.tile([2 * L, NPR, B], BF16, tag="ycp")
                for blk in range(NPR // 8):
                    pv = vps.tile([2 * L, 8, B], F32, tag="mv")
                    for k in range(8):
                        pr = blk * 8 + k
                        ee = e0 + 2 * pr
                        nc.tensor.matmul(
                            pv[0:64, k, :], lhsT=kTtmp[:, :, 2 * pr],
                            rhs=tT[:, :, 2 * pr],
                            start=True, stop=True,
                        )
                        nc.tensor.matmul(
                            pv[64:128, k, :], lhsT=kTtmp[:, :, 2 * pr + 1],
                            rhs=tT[:, :, 2 * pr + 1],
                            start=True, stop=True,
                        )
                    if blk % 2 == 0:
                        nc.scalar.copy(ycp[:, blk * 8:(blk + 1) * 8, :],
                                       pv[:, :, :])
                    else:
                        nc.vector.tensor_copy(
                            ycp[:, blk * 8:(blk + 1) * 8, :], pv[:, :, :])

                # PE transpose per (parity j, b): [g, pair] -> [pair, g]
                yfE = yfpool.tile([NPR, B, G], BF16, tag="yfE")
                yfO = yfpool.tile([NPR, B, G], BF16, tag="yfO")
                for j, yf in ((0, yfE), (1, yfO)):
                    for b8 in range(B // 8):
                        pt = tps.tile([NPR, 8, G], BF16, tag="tr")
                        for i in range(8):
                            b = b8 * 8 + i
                            nc.tensor.transpose(
                                pt[:, i, :],
                                ycp[j * 64:(j + 1) * 64, :, b],
                                ident[j * 64:(j + 1) * 64, :],
                            )
                        if (b8 + j) % 2 == 0:
                            nc.vector.tensor_copy(
                                yf[:, b8 * 8:(b8 + 1) * 8, :], pt[:, :, :])
                        else:
                            nc.scalar.copy(
                                yf[:, b8 * 8:(b8 + 1) * 8, :], pt[:, :, :])
                nc.sync.dma_start(
                    out=yE_d[ch * NPR:(ch + 1) * NPR, :], in_=yfE[:, :, :])
                nc.sync.dma_start(
                    out=yO_d[ch * NPR:(ch + 1) * NPR, :], in_=yfO[:, :, :])

        # ---- phase 2: scatter, two chains (even/odd edges), PSUM acc ----
        NEC = (EL // 2) // 128          # 8 pair-chunks of 128
        with (
            tc.tile_pool(name="p2c", bufs=1) as p2c,
            tc.tile_pool(name="p2rhs", bufs=12) as p2r,
            tc.tile_pool(name="acc_ps", bufs=8, space="PSUM") as aps,
        ):
            incTE_sb = p2c.tile([128, NEC, NPAD], BF16)     # 8KB/part
            nc.sync.dma_start(
                out=incTE_sb[:, :, :],
                in_=incTE_d[:, :].rearrange("(c e) n -> e c n", c=NEC),
            )
            incTO_sb = p2c.tile([128, NEC, NPAD], BF16)     # 8KB/part
            nc.sync.dma_start(
                out=incTO_sb[:, :, :],
                in_=incTO_d[:, :].rearrange("(c e) n -> e c n", c=NEC),
            )
            for nj in range(BG // 512):
                paccs = [aps.tile([128, 8, G], F32, tag="acc", name=f"acc{nj}_{m}")
                         for m in range(NQ)]
                for ec in range(NEC):
                    rtE = p2r.tile([128, 512], BF16, tag="rhsE")
                    nc.sync.dma_start(
                        out=rtE[:, :],
                        in_=yE_d[ec * 128:(ec + 1) * 128,
                                 nj * 512:(nj + 1) * 512],
                    )
                    rtO = p2r.tile([128, 512], BF16, tag="rhsO")
                    nc.sync.dma_start(
                        out=rtO[:, :],
                        in_=yO_d[ec * 128:(ec + 1) * 128,
                                 nj * 512:(nj + 1) * 512],
                    )
                    for m in range(NQ):
                        nc.tensor.matmul(
                            paccs[m][:, :, :],
                            lhsT=incTE_sb[:, ec, m * 128:(m + 1) * 128],
                            rhs=rtE[:, :],
                            start=(ec == 0), stop=False,
                        )
                        nc.tensor.matmul(
                            paccs[m][:, :, :],
                            lhsT=incTO_sb[:, ec, m * 128:(m + 1) * 128],
                            rhs=rtO[:, :],
                            start=False, stop=(ec == NEC - 1),
                        )
                for m in range(NQ):
                    rows = min(N - m * 128, 128)
                    ot = p2r.tile([128, 8, G], BF16, tag="ostage",
                                  name=f"ost{nj}_{m}")
                    if m % 2 == 0:
                        nc.vector.tensor_copy(ot[:rows, :, :],
                                              paccs[m][:rows, :, :])
                    else:
                        nc.scalar.copy(ot[:rows, :, :], paccs[m][:rows, :, :])
                    nc.sync.dma_start(
                        out=out_d[nj * 8:(nj + 1) * 8,
                                  m * 128:m * 128 + rows, :].transpose(
                                      [1, 0, 2]),
                        in_=ot[:rows, :, :],
                    )
    nc.compile()
    return nc


def kernel(x, incidence, ef, W1, b1, W2, b2, b_gc):
    global last_results
    x = np.asarray(x, dtype=np.float32)
    incidence = np.asarray(incidence, dtype=np.float32)
    ef = np.asarray(ef, dtype=np.float32)
    W1 = np.asarray(W1, dtype=np.float32)
    b1 = np.asarray(b1, dtype=np.float32)
    W2 = np.asarray(W2, dtype=np.float32)
    b2 = np.asarray(b2, dtype=np.float32)
    b_gc = np.asarray(b_gc, dtype=np.float32)

    if "nc" not in _CACHE:
        _CACHE["nc"] = _build()
    nc = _CACHE["nc"]

    bf = ml_dtypes.bfloat16
    xT = np.ascontiguousarray(
        x.transpose(1, 0, 2).reshape(N, B * L)).astype(bf)
    inc_bf = incidence.astype(bf)
    incT_bf = np.ascontiguousarray(incidence.T).astype(bf)
    efT = np.ascontiguousarray(ef.T).astype(bf)
    b1c = np.ascontiguousarray(b1.reshape(H, 1))
    W2_bf = W2.astype(bf)
    b2T = np.ascontiguousarray(b2.reshape(G * L // H, H).T)

    pad = EL - ELR
    in_maps = []
    for c in range(NCORES):
        es = slice(c * ELR, (c + 1) * ELR)
        incT_pad = np.pad(incT_bf[es, :], ((0, pad), (0, NPAD - N)))
        in_maps.append({
            "xT": xT,
            "inc": np.ascontiguousarray(
                np.pad(inc_bf[:, es], ((0, 0), (0, pad)))),
            "incTE": np.ascontiguousarray(incT_pad[0::2, :]),
            "incTO": np.ascontiguousarray(incT_pad[1::2, :]),
            "efT": np.ascontiguousarray(
                np.pad(efT[:, es], ((0, 0), (0, pad)))),
            "W1": W1.astype(bf), "b1": b1c, "W2": W2_bf, "b2T": b2T,
        })

    import os
    trace = bool(int(os.environ.get("KERNEL_TRACE", "0")))
    last_results = run_bass_kernel_spmd(
        nc, in_maps, list(range(NCORES)), trace=trace)
    partial = np.zeros((B, N, G), np.float32)
    for r in last_results.results:
        partial += np.asarray(r["out"], dtype=np.float32)
    out = np.maximum(partial + b_gc.reshape(1, 1, G), 0.0)
    return out.reshape(B, N * G).astype(np.float32)


# revision 20
# speedup vs baseline: 1.3760x; 1.0377x over previous
"""GNN message-passing kernel for 8 Trainium2 NeuronCores.

Math (per reference):
  h   = relu(ef @ W1 + b1)                      [E, H]
  K   = (h @ W2 + b2).reshape(E, G, L)          per-edge [G, L] kernels
  t   = einsum('bnl,ne->bel', x, inc)           gather nodes->edges
  y   = einsum('egl,bel->beg', K, t)            per-edge matvec
  out = relu(einsum('ne,beg->bng', inc, y) + b_gc).reshape(B, N*G)

Distribution: shard E across the 8 cores (2000 edges each, padded to
2048 with zero-incidence edges); host sums the per-core scatter
partials, then bias + relu.

v4 notes:
  - mlp2 drains land contiguous in kTtmp[l,g,e]; the idle GpSimd engine
    re-layouts to kT[l,e,g] so the per-edge matvec gets contiguous
    64-col weight loads.
  - matvec packs edge PAIRS into the PE array via column tile_position
    (even edge -> psum partitions 0:64, odd edge -> 64:128), so weight
    loads for one half overlap matmuls on the other half.
  - y transposes split by edge parity into Y_even / Y_odd DRAM stages;
    phase-2 scatter runs two interleaved accumulation chains with
    host-deinterleaved incidence (incT[0::2], incT[1::2]).
  - phase-2 lhsT tiles padded to 128 cols; output staged bf16.
"""

import numpy as np
import ml_dtypes

import concourse.bass as bass
from concourse import bacc
import concourse.mybir as mybir
import concourse.tile as tile
from concourse.bass_utils import run_bass_kernel_spmd
from concourse.masks import make_identity

B, N, E, L, G, F, H = 64, 500, 16000, 64, 64, 8, 128
NCORES = 8
ELR = E // NCORES       # 2000 real edges per core
EL = 2048               # padded; pad edges have zero incidence columns
ECH = 256               # edge chunk (phase 1)
NCH = EL // ECH         # 8 chunks
NPR = ECH // 2          # 128 edge-pairs per chunk
NP = 125                # nodes per n-chunk (500 = 4*125)
NQ = 4                  # n-chunks
NPAD = 512              # padded node count for phase-2 FWL tiles
BG = B * G              # 4096
F32 = mybir.dt.float32
BF16 = mybir.dt.bfloat16
RELU = mybir.ActivationFunctionType.Relu
IDENT = mybir.ActivationFunctionType.Identity

_CACHE = {}
last_results = None     # BassKernelResults of the most recent run (for test.py)


def _build():
    nc = bacc.Bacc("TRN2", target_bir_lowering=False)
    xT_d = nc.declare_dram_parameter("xT", [N, B * L], BF16, isOutput=False)
    inc_d = nc.declare_dram_parameter("inc", [N, EL], BF16, isOutput=False)
    incTE_d = nc.declare_dram_parameter("incTE", [EL // 2, NPAD], BF16,
                                        isOutput=False)
    incTO_d = nc.declare_dram_parameter("incTO", [EL // 2, NPAD], BF16,
                                        isOutput=False)
    efT_d = nc.declare_dram_parameter("efT", [F, EL], BF16, isOutput=False)
    W1_d = nc.declare_dram_parameter("W1", [F, H], BF16, isOutput=False)
    b1_d = nc.declare_dram_parameter("b1", [H, 1], F32, isOutput=False)
    W2_d = nc.declare_dram_parameter("W2", [H, G * L], BF16, isOutput=False)
    b2T_d = nc.declare_dram_parameter("b2T", [H, G * L // H], F32, isOutput=False)
    out_d = nc.declare_dram_parameter("out", [B, N, G], BF16, isOutput=True)
    yE_d = nc.dram_tensor("YstageE", [EL // 2, BG], BF16)
    yO_d = nc.dram_tensor("YstageO", [EL // 2, BG], BF16)

    with tile.TileContext(nc) as tc, tc.tile_pool(name="const", bufs=1) as cpool:
        with tc.tile_pool(name="h_ps", bufs=2, space="PSUM") as hps:
            # ---- persistent tiles ----
            W1_sb = cpool.tile([F, H], BF16)
            nc.sync.dma_start(out=W1_sb[:, :], in_=W1_d[:, :])
            b1_sb = cpool.tile([H, 1], F32)
            nc.sync.dma_start(out=b1_sb[:, :], in_=b1_d[:, :])
            W2_sb = cpool.tile([H, G * L], BF16)            # 8KB/part
            nc.sync.dma_start(out=W2_sb[:, :], in_=W2_d[:, :])
            b2T_sb = cpool.tile([H, G * L // H], F32)
            nc.sync.dma_start(out=b2T_sb[:, :], in_=b2T_d[:, :])
            efT_sb = cpool.tile([F, EL], BF16)
            nc.sync.dma_start(out=efT_sb[:, :], in_=efT_d[:, :])
            hT_sb = cpool.tile([H, EL], BF16)               # 4KB/part
            xT_sb = cpool.tile([NP, NQ, B * L], BF16)       # 32KB/part
            inc0_sb = cpool.tile([NP, NQ, ECH], BF16)       # chunk-0 inc early
            nc.sync.dma_start(
                out=inc0_sb[:, :, :],
                in_=inc_d[:, 0:ECH].rearrange("(q n) e -> n q e", q=NQ),
            )
            for q in range(NQ):
                nc.sync.dma_start(
                    out=xT_sb[:, q, :],
                    in_=xT_d[q * NP:(q + 1) * NP, :],
                )

            # ---- mlp1: hT = relu(W1.T @ efT + b1), all edges upfront ----
            for c in range(4):
                ph = hps.tile([H, 512], F32)
                nc.tensor.matmul(
                    ph[:, :], lhsT=W1_sb[:, :],
                    rhs=efT_sb[:, c * 512:(c + 1) * 512],
                    start=True, stop=True,
                )
                nc.scalar.activation(
                    hT_sb[:, c * 512:(c + 1) * 512], ph[:, :], RELU,
                    bias=b1_sb[:, 0:1],
                )

        # ---- phase 1 ----
        with (
            tc.tile_pool(name="ktt", bufs=2) as kttpool,
            tc.tile_pool(name="tt", bufs=1) as ttpool,
            tc.tile_pool(name="inct", bufs=2) as incpool,
            tc.tile_pool(name="ycp", bufs=1) as ycppool,
            tc.tile_pool(name="yfin", bufs=1) as yfpool,
            tc.tile_pool(name="tid", bufs=1) as idpool,
            tc.tile_pool(name="mlp2_ps", bufs=2, space="PSUM") as mps,
            tc.tile_pool(name="gat_ps", bufs=2, space="PSUM") as gps,
            tc.tile_pool(name="mv_ps", bufs=2, space="PSUM") as vps,
            tc.tile_pool(name="tr_ps", bufs=2, space="PSUM") as tps,
        ):
            # identity blocks on both partition halves (for j=1 transposes)
            ident = idpool.tile([2 * L, L], BF16)
            make_identity(nc, ident[0:L, :])
            nc.vector.tensor_copy(ident[L:2 * L, :], ident[0:L, :])
            for ch in range(NCH):
                e0 = ch * ECH
                # mlp2 -> kTtmp[l, g, e] bf16 (+b2)
                kTtmp = kttpool.tile([L, G, ECH], BF16, tag="ktt")
                for mc in range(32):
                    pm = mps.tile([H, ECH], F32, tag="m2")
                    nc.tensor.matmul(
                        pm[:, :], lhsT=W2_sb[:, mc * H:(mc + 1) * H],
                        rhs=hT_sb[:, e0:e0 + ECH], start=True, stop=True,
                    )
                    for par in (0, 1):
                        src = pm[par * 64:(par + 1) * 64, :]
                        dst = kTtmp[:, 2 * mc + par, :]
                        bias = b2T_sb[par * 64:(par + 1) * 64, mc:mc + 1]
                        if mc % 2 == 0:
                            nc.scalar.activation(dst, src, IDENT, bias=bias)
                        else:
                            nc.vector.tensor_scalar_add(dst, src, bias)

                # gather -> tT[l, b, e] bf16
                if ch == 0:
                    inc_t = inc0_sb
                else:
                    inc_t = incpool.tile([NP, NQ, ECH], BF16, tag="inc")
                    nc.sync.dma_start(
                        out=inc_t[:, :, :],
                        in_=inc_d[:, e0:e0 + ECH].rearrange(
                            "(q n) e -> n q e", q=NQ),
                    )
                tT = ttpool.tile([L, B, ECH], BF16, tag="tt")
                for bp2 in range(B // 4):
                    pg = gps.tile([2 * L, 2, ECH], F32, tag="g")
                    for s in (0, 1):
                        bp = 2 * bp2 + s
                        for q in range(NQ):
                            nc.tensor.matmul(
                                pg[:, s, :],
                                lhsT=xT_sb[:, q, bp * 128:(bp + 1) * 128],
                                rhs=inc_t[:, q, :],
                                start=(q == 0), stop=(q == NQ - 1),
                            )
                    for par in (0, 1):
                        src = pg[par * 64:(par + 1) * 64, :, :]
                        b0 = 4 * bp2 + par
                        dst = tT[:, b0:b0 + 3:2, :]
                        if bp2 % 2 == 0:
                            nc.scalar.copy(dst, src)
                        else:
                            nc.vector.tensor_copy(dst, src)

                # paired matvec: even edge -> psum rows 0:64 (col tile 0),
                # odd edge -> rows 64:128 (col tile 64); 8 pairs per bank
                ycp = ycppool.tile([2 * L, NPR, B], BF16, tag="ycp")
                for blk in range(NPR // 8):
                    pv = vps.tile([2 * L, 8, B], F32, tag="mv")
                    for k in range(8):
                        pr = blk * 8 + k
                        ee = e0 + 2 * pr
                        nc.tensor.matmul(
                            pv[0:64, k, :], lhsT=kTtmp[:, :, 2 * pr],
                            rhs=tT[:, :, 2 * pr],
                            start=True, stop=True,
                        )
                        nc.tensor.matmul(
                            pv[64:128, k, :], lhsT=kTtmp[:, :, 2 * pr + 1],
                            rhs=tT[:, :, 2 * pr + 1],
                            start=True, stop=True,
                        )
                    if blk % 2 == 0:
                        nc.scalar.copy(ycp[:, blk * 8:(blk + 1) * 8, :],
                                       pv[:, :, :])
                    else:
                        nc.vector.tensor_copy(
                            ycp[:, blk * 8:(blk + 1) * 8, :], pv[:, :, :])

                # PE transpose per (parity j, b): [g, pair] -> [pair, g]
                yfE = yfpool.tile([NPR, B, G], BF16, tag="yfE")
                yfO = yfpool.tile([NPR, B, G], BF16, tag="yfO")
                for j, yf in ((0, yfE), (1, yfO)):
                    for b8 in range(B // 8):
                        pt = tps.tile([NPR, 8, G], BF16, tag="tr")
                        for i in range(8):
                            b = b8 * 8 + i
                            nc.tensor.transpose(
                                pt[:, i, :],
                                ycp[j * 64:(j + 1) * 64, :, b],
                                ident[j * 64:(j + 1) * 64, :],
                            )
                        if (b8 + j) % 2 == 0:
                            nc.vector.tensor_copy(
                                yf[:, b8 * 8:(b8 + 1) * 8, :], pt[:, :, :])
                        else:
                            nc.scalar.copy(
                                yf[:, b8 * 8:(b8 + 1) * 8, :], pt[:, :, :])
                nc.sync.dma_start(
                    out=yE_d[ch * NPR:(ch + 1) * NPR, :], in_=yfE[:, :, :])
                nc.sync.dma_start(
                    out=yO_d[ch * NPR:(ch + 1) * NPR, :], in_=yfO[:, :, :])

        # ---- phase 2: scatter, two chains (even/odd edges), PSUM acc ----
        NEC = (EL // 2) // 128          # 8 pair-chunks of 128
        with (
            tc.tile_pool(name="p2c", bufs=1) as p2c,
            tc.tile_pool(name="p2rhs", bufs=12) as p2r,
            tc.tile_pool(name="acc_ps", bufs=8, space="PSUM") as aps,
        ):
            incTE_sb = p2c.tile([128, NEC, NPAD], BF16)     # 8KB/part
            nc.sync.dma_start(
                out=incTE_sb[:, :, :],
                in_=incTE_d[:, :].rearrange("(c e) n -> e c n", c=NEC),
            )
            incTO_sb = p2c.tile([128, NEC, NPAD], BF16)     # 8KB/part
            nc.sync.dma_start(
                out=incTO_sb[:, :, :],
                in_=incTO_d[:, :].rearrange("(c e) n -> e c n", c=NEC),
            )
            for nj in range(BG // 512):
                paccs = [aps.tile([128, 8, G], F32, tag="acc", name=f"acc{nj}_{m}")
                         for m in range(NQ)]
                for ec in range(NEC):
                    rtE = p2r.tile([128, 512], BF16, tag="rhsE")
                    nc.sync.dma_start(
                        out=rtE[:, :],
                        in_=yE_d[ec * 128:(ec + 1) * 128,
                                 nj * 512:(nj + 1) * 512],
                    )
                    rtO = p2r.tile([128, 512], BF16, tag="rhsO")
                    nc.sync.dma_start(
                        out=rtO[:, :],
                        in_=yO_d[ec * 128:(ec + 1) * 128,
                                 nj * 512:(nj + 1) * 512],
                    )
                    for m in range(NQ):
                        nc.tensor.matmul(
                            paccs[m][:, :, :],
                            lhsT=incTE_sb[:, ec, m * 128:(m + 1) * 128],
                            rhs=rtE[:, :],
                            start=(ec == 0), stop=False,
                        )
                        nc.tensor.matmul(
                            paccs[m][:, :, :],
                            lhsT=incTO_sb[:, ec, m * 128:(m + 1) * 128],
                            rhs=rtO[:, :],
                            start=False, stop=(ec == NEC - 1),
                        )
                for m in range(NQ):
                    rows = min(N - m * 128, 128)
                    ot = p2r.tile([128, 8, G], BF16, tag="ostage",
                                  name=f"ost{nj}_{m}")
                    if m % 2 == 0:
                        nc.vector.tensor_copy(ot[:rows, :, :],
                                              paccs[m][:rows, :, :])
                    else:
                        nc.scalar.copy(ot[:rows, :, :], paccs[m][:rows, :, :])
                    nc.sync.dma_start(
                        out=out_d[nj * 8:(nj + 1) * 8,
                                  m * 128:m * 128 + rows, :].transpose(
                                      [1, 0, 2]),
                        in_=ot[:rows, :, :],
                    )
    nc.compile()
    return nc


def kernel(x, incidence, ef, W1, b1, W2, b2, b_gc):
    global last_results
    x = np.asarray(x, dtype=np.float32)
    incidence = np.asarray(incidence, dtype=np.float32)
    ef = np.asarray(ef, dtype=np.float32)
    W1 = np.asarray(W1, dtype=np.float32)
    b1 = np.asarray(b1, dtype=np.float32)
    W2 = np.asarray(W2, dtype=np.float32)
    b2 = np.asarray(b2, dtype=np.float32)
    b_gc = np.asarray(b_gc, dtype=np.float32)

    if "nc" not in _CACHE:
        _CACHE["nc"] = _build()
    nc = _CACHE["nc"]

    bf = ml_dtypes.bfloat16
    xT = np.ascontiguousarray(
        x.transpose(1, 0, 2).reshape(N, B * L)).astype(bf)
    inc_bf = incidence.astype(bf)
    incT_bf = np.ascontiguousarray(incidence.T).astype(bf)
    efT = np.ascontiguousarray(ef.T).astype(bf)
    b1c = np.ascontiguousarray(b1.reshape(H, 1))
    W2_bf = W2.astype(bf)
    b2T = np.ascontiguousarray(b2.reshape(G * L // H, H).T)

    pad = EL - ELR
    in_maps = []
    for c in range(NCORES):
        es = slice(c * ELR, (c + 1) * ELR)
        incT_pad = np.pad(incT_bf[es, :], ((0, pad), (0, NPAD - N)))
        in_maps.append({
            "xT": xT,
            "inc": np.ascontiguousarray(
                np.pad(inc_bf[:, es], ((0, 0), (0, pad)))),
            "incTE": np.ascontiguousarray(incT_pad[0::2, :]),
            "incTO": np.ascontiguousarray(incT_pad[1::2, :]),
            "efT": np.ascontiguousarray(
                np.pad(efT[:, es], ((0, 0), (0, pad)))),
            "W1": W1.astype(bf), "b1": b1c, "W2": W2_bf, "b2T": b2T,
        })

    import os
    trace = bool(int(os.environ.get("KERNEL_TRACE", "0")))
    last_results = run_bass_kernel_spmd(
        nc, in_maps, list(range(NCORES)), trace=trace)
    partial = np.zeros((B, N, G), np.float32)
    for r in last_results.results:
        partial += np.asarray(r["out"], dtype=np.float32)
    out = np.maximum(partial + b_gc.reshape(1, 1, G), 0.0)
    return out.reshape(B, N * G).astype(np.float32)
